# revision 1
# baseline (speedup 1.0000x reference)
"""Trainium2 Bass kernel for nn_MultiHeadTokenAttention.

Reference computation (per batch element b):
    q = ini_q @ Wq.T + bq                      [Q, H] -> heads [Q, 16, 64]
    k = ini_k @ Wk.T + bk                      [S, T, H]
    v = ini_k @ Wv.T + bv
    scores[h,q,s,t] = (q_h . k_h) / 8,  + mask*-1e4, softmax over t
    res[q,s,:] = concat_h(sum_t alpha * v_h)   [Q, S, H]
    res = res @ Wo.T + bo;  LayerNorm(res) * gamma + beta

Sharding: batch-parallel, one batch element per NeuronCore (8 cores, no
collectives).  Per core everything streams over 16 chunks of 4 s-values;
softmax is local to s so nothing large persists in SBUF.

Layout strategy per chunk (4 s-values = 512 rows of X):
  X   [128, 1024] x4   (natural rows, contiguous DMA)
  X^T [128, 512] x8    (PE transpose; written f32r)
  K^T [128, 512] x8    hd on partitions   (lhsT=Wk^T chunk, rhs=X^T)
  V   [128, 1024] x4   t on partitions    (lhsT=X^T chunk, rhs=Wv^T)
  scores psum [128(4 heads x 32 q), 512(4 s x 128 t)]  (4 matmuls/bank)
  softmax: +(-1e4*mask) bcast, exp with fused row-sum (accum_out),
           no max-subtraction needed (|scores| <= ~10), deferred division
  alpha^T via PE transpose -> attn.V with V stationary -> res^T [hd, (s,q)]
  O-proj consumes res^T directly as its stationary operand (no transpose),
  LayerNorm over H on [128(s,q), 1024] rows, strided DMA to out[q,s,:].

The 1/sqrt(head_dim) scale is folded into Wq on the host (exact, power of 2).
Matmuls run in float32r (fast fp32 mode, 1 cyc/row at N>=256) by default;
the attention-value matmuls and alpha transposes stay plain fp32 (same speed
at their shapes, better accuracy).
"""

import os
import sys

for _p in ("/opt/trn_rl_repo", "/root/.axon_site/_ro/trn_rl_repo"):
    if os.path.isdir(_p) and _p not in sys.path:
        sys.path.insert(0, _p)

import numpy as np

B, Q, S, T, H = 8, 32, 64, 128, 1024
HEADS, D = 16, 64
ST = S * T           # 8192 rows of ini_k per batch element
NCORES = 8
NG = 16              # chunks per core (4 s-values each)
EPS = 1e-12

_BUILD_CACHE = {}


def _build(mm_fast=True, bias_kq=False, bias_v=False, bias_o=False,
           gamma_beta=False, loop=1, debug_taps=False, stages=9):
    """Build + compile the Bass program. Returns the Bacc object."""
    import concourse.mybir as mybir
    from concourse import bacc
    from concourse.tile import TileContext
    from concourse.masks import make_identity

    f32 = mybir.dt.float32
    mdt = mybir.dt.float32r if mm_fast else f32
    ADD = mybir.AluOpType.add
    SUB = mybir.AluOpType.subtract
    MULT = mybir.AluOpType.mult
    AXX = mybir.AxisListType.X
    EXP = mybir.ActivationFunctionType.Exp
    SQUARE = mybir.ActivationFunctionType.Square
    SQRT = mybir.ActivationFunctionType.Sqrt

    nc = bacc.Bacc("TRN2", target_bir_lowering=False, debug=False,
                   num_devices=NCORES)

    xq_d = nc.dram_tensor("xq", [Q, H], f32, kind="ExternalInput")
    xk_d = nc.dram_tensor("xk", [ST, H], f32, kind="ExternalInput")
    mneg_d = nc.dram_tensor("mneg", [S, T], f32, kind="ExternalInput")
    wqt_d = nc.dram_tensor("wqt", [H, H], f32, kind="ExternalInput")
    wkt_d = nc.dram_tensor("wkt", [H, H], f32, kind="ExternalInput")
    wvt_d = nc.dram_tensor("wvt", [H, H], f32, kind="ExternalInput")
    wot_d = nc.dram_tensor("wot", [H, H], f32, kind="ExternalInput")
    bq_d = nc.dram_tensor("bqr", [8, 128], f32, kind="ExternalInput")
    bk_d = nc.dram_tensor("bkr", [8, 128], f32, kind="ExternalInput")
    bv_d = nc.dram_tensor("bvr", [1, H], f32, kind="ExternalInput")
    bo_d = nc.dram_tensor("bor", [1, H], f32, kind="ExternalInput")
    gam_d = nc.dram_tensor("gam", [1, H], f32, kind="ExternalInput")
    bet_d = nc.dram_tensor("bet", [1, H], f32, kind="ExternalInput")
    zpad_d = nc.dram_tensor("zpad", [128, 64], f32, kind="ExternalInput")
    out_d = nc.dram_tensor("out", [Q, S, H], f32, kind="ExternalOutput")
    dbg = {}
    if debug_taps:
        for nm, shp in (("xqt0", [128, Q]), ("qnat", [Q, H]),
                        ("qt0", [128, 2 * Q]), ("kt0", [128, 512]),
                        ("ex0", [64, 512]), ("at0", [128, 256]),
                        ("rt0", [128, 512]), ("osb0", [128, H])):
            dbg[nm] = nc.dram_tensor("dbg_" + nm, shp, f32,
                                     kind="ExternalOutput")

    with TileContext(nc) as tc:
        with tc.tile_pool(name="wts", bufs=1) as wpool, \
             tc.tile_pool(name="ppxt", bufs=2, space="PSUM") as ppxt, \
             tc.tile_pool(name="ppmm", bufs=4, space="PSUM") as ppmm:

            # ---------------- preamble: constants + weights ----------------
            ident = wpool.tile([128, 128], f32, name="ident")
            make_identity(nc, ident)
            eps_sb = wpool.tile([128, 1], f32, name="eps_sb")
            nc.vector.memset(eps_sb[:], EPS)

            wk_t, wv_t, wo_t = [], [], []
            for c in range(8):
                wkc = wpool.tile([128, H], mdt, name=f"wk{c}")
                wvc = wpool.tile([128, H], mdt, name=f"wv{c}")
                woc = wpool.tile([128, H], mdt, name=f"wo{c}")
                nc.gpsimd.dma_start(wkc[:], wkt_d[128 * c:128 * (c + 1), :])
                nc.gpsimd.dma_start(wvc[:], wvt_d[128 * c:128 * (c + 1), :])
                nc.gpsimd.dma_start(woc[:], wot_d[128 * c:128 * (c + 1), :])
                wk_t.append(wkc)
                wv_t.append(wvc)
                wo_t.append(woc)

            if bias_kq:
                bq_sb = wpool.tile([128, 8], f32, name="bq_sb")
                bk_sb = wpool.tile([128, 8], f32, name="bk_sb")
                nc.sync.dma_start(bq_sb[:], bq_d[:].rearrange("m p -> p m"))
                nc.sync.dma_start(bk_sb[:], bk_d[:].rearrange("m p -> p m"))
            if bias_v or bias_o:
                ones_sb = wpool.tile([1, 128], mdt, name="ones_sb")
                nc.vector.memset(ones_sb[:], 1.0)
            if bias_v:
                bv_sb = wpool.tile([1, H], mdt, name="bv_sb")
                nc.gpsimd.dma_start(bv_sb[:], bv_d[:])
            if bias_o:
                bo_sb = wpool.tile([1, H], mdt, name="bo_sb")
                nc.gpsimd.dma_start(bo_sb[:], bo_d[:])
            if gamma_beta:
                gam_sb = wpool.tile([128, H], f32, name="gam_sb")
                bet_sb = wpool.tile([128, H], f32, name="bet_sb")
                nc.sync.dma_start(
                    gam_sb[:], gam_d[0, :].partition_broadcast(128))
                nc.sync.dma_start(
                    bet_sb[:], bet_d[0, :].partition_broadcast(128))

            # Q path:  q = xq @ (Wq^T/8)  ->  qtpad_m [128 hd(2 heads), 64]
            # block-diagonal: cols 0:32 head 2m (rows 0:64), cols 32:64 head
            # 2m+1 (rows 64:128); zeros elsewhere so one matmul with K=128
            # computes both heads' scores without cross terms (f32r cannot
            # column-tile, so the out partition base must stay 0).
            qt_t = [wpool.tile([128, 2 * Q], mdt, name=f"qt{m}")
                    for m in range(8)]
            for m in range(8):
                nc.gpsimd.dma_start(qt_t[m][:], zpad_d[:])
            with tc.tile_pool(name="qtmp", bufs=2) as qtmp:
                xq_sb = qtmp.tile([Q, H], f32, name="xq_sb", bufs=1)
                nc.sync.dma_start(xq_sb[:], xq_d[:])
                xqt = []
                for c in range(8):
                    pq = ppxt.tile([128, 512], f32, name="pq", tag="xt")
                    nc.tensor.transpose(
                        pq[:, 0:Q], xq_sb[:, 128 * c:128 * (c + 1)],
                        ident[0:Q, 0:Q])
                    xqtc = qtmp.tile([128, Q], mdt, name=f"xqt{c}", bufs=1)
                    nc.scalar.copy(xqtc[:], pq[:, 0:Q])
                    xqt.append(xqtc)
                    if debug_taps and c == 0:
                        nc.sync.dma_start(dbg["xqt0"][:],
                                          xqtc[:].bitcast(f32))
                q_sb = qtmp.tile([Q, H], f32, name="q_sb", bufs=1)
                for n in range(2):
                    pqn = ppmm.tile([128, 512], f32, name="pqn", tag="mm")
                    for c in range(8):
                        wq_c = qtmp.tile([128, 512], mdt, name="wq_c")
                        nc.gpsimd.dma_start(
                            wq_c[:],
                            wqt_d[128 * c:128 * (c + 1),
                                  512 * n:512 * (n + 1)])
                        nc.tensor.matmul(
                            pqn[0:Q, :], xqt[c][:], wq_c[:],
                            start=(c == 0), stop=(c == 7))
                    nc.scalar.copy(q_sb[:, 512 * n:512 * (n + 1)], pqn[0:Q, :])
                if debug_taps:
                    nc.sync.dma_start(dbg["qnat"][:], q_sb[:])
                for m in range(8):
                    pqt = ppxt.tile([128, 512], f32, name="pqt", tag="xt")
                    nc.tensor.transpose(
                        pqt[:, 0:Q], q_sb[:, 128 * m:128 * (m + 1)],
                        ident[0:Q, 0:Q])
                    if bias_kq:
                        nc.vector.tensor_scalar(
                            qt_t[m][0:64, 0:Q], pqt[0:64, 0:Q],
                            bq_sb[0:64, m:m + 1], None, ADD)
                        nc.vector.tensor_scalar(
                            qt_t[m][64:128, Q:2 * Q], pqt[64:128, 0:Q],
                            bq_sb[64:128, m:m + 1], None, ADD)
                    else:
                        nc.scalar.copy(qt_t[m][0:64, 0:Q], pqt[0:64, 0:Q])
                        nc.scalar.copy(qt_t[m][64:128, Q:2 * Q],
                                       pqt[64:128, 0:Q])

            if debug_taps:
                nc.sync.dma_start(dbg["qt0"][:], qt_t[0][:].bitcast(f32))

            # ---------------- main per-chunk pipeline ----------------
            with tc.tile_pool(name="io", bufs=1) as iop, \
                 tc.tile_pool(name="io2", bufs=2) as iop2, \
                 tc.tile_pool(name="sm", bufs=1) as smp, \
                 tc.tile_pool(name="sm2", bufs=2) as smp2, \
                 tc.tile_pool(name="ppat", bufs=1, space="PSUM") as ppat, \
                 tc.tile_pool(name="ppr", bufs=1, space="PSUM") as ppr:

                def emit_chunk(g):
                    # 1. load X (4 tiles) + mask tile
                    x_t = []
                    for j in range(4):
                        xj = iop.tile([128, H], f32, name=f"x{j}")
                        nc.sync.dma_start(
                            xj[:],
                            xk_d[512 * g + 128 * j:512 * g + 128 * (j + 1), :])
                        x_t.append(xj)
                    mt = iop2.tile([128, 512], f32, name="mt")
                    nc.sync.dma_start(
                        mt[:],
                        mneg_d[4 * g:4 * (g + 1), :]
                        .rearrange("s t -> (s t)").partition_broadcast(128))

                    # 2. X^T via PE transpose (fp32, 2 cyc/row)
                    xt_t = []
                    for c in range(8):
                        pxt = ppxt.tile([128, 512], f32, name="pxt", tag="xt")
                        for j in range(4):
                            nc.tensor.transpose(
                                pxt[:, 128 * j:128 * (j + 1)],
                                x_t[j][:, 128 * c:128 * (c + 1)], ident[:])
                        xtc = iop.tile([128, 512], mdt, name=f"xt{c}")
                        nc.scalar.copy(xtc[:], pxt[:])
                        xt_t.append(xtc)

                    if stages < 3:
                        nc.sync.dma_start(out_d[:, 4 * g:4 * (g + 1), :]
                                          .rearrange("q s h -> s q h"),
                                          xt_t[0][:].bitcast(f32)
                                          .broadcast_to([128, 512, 8])
                                          .rearrange("p a b -> p (a b)")[:, 0:4096].rearrange("p (s h) -> p s h", s=4)) if False else None
                        osb_stub = iop2.tile([128, H], f32, name="osb")
                        nc.vector.tensor_copy(osb_stub[:, 0:512], xt_t[0][:].bitcast(f32))
                        nc.vector.tensor_copy(osb_stub[:, 512:1024], xt_t[7][:].bitcast(f32))
                        nc.sync.dma_start(
                            out_d[:, 4 * g:4 * (g + 1), :]
                            .rearrange("q s h -> s q h"), osb_stub[:])
                        return

                    # 3+5-7. per head-pair m (heads 2m, 2m+1): K^T proj ->
                    # scores ps_m [64 (2 x 32q), 512 (4s x 128t)] -> masked
                    # exp with fused row sums (no max subtraction needed:
                    # |scores| <= ~10).  kt tiles die right after their
                    # scores matmul, so they rotate through 3 shared slots.
                    sums = smp2.tile([64, 32], f32, name="sums")
                    ex_t = [smp.tile([64, 512], f32, name=f"ex{m}")
                            for m in range(8)]
                    for m in range(8):
                        pk = ppmm.tile([128, 512], f32, name="pk", tag="mm")
                        for c in range(8):
                            nc.tensor.matmul(
                                pk[:], wk_t[c][:, 128 * m:128 * (m + 1)],
                                xt_t[c][:], start=(c == 0), stop=(c == 7))
                        ktm = iop.tile([128, 512], mdt, name="ktm", tag="kt",
                                       bufs=3)
                        if bias_kq:
                            nc.vector.tensor_scalar(
                                ktm[:], pk[:], bk_sb[:, m:m + 1], None, ADD)
                        else:
                            nc.vector.tensor_copy(ktm[:], pk[:])
                        if debug_taps and g == 0 and m == 0:
                            nc.sync.dma_start(dbg["kt0"][:],
                                              ktm[:].bitcast(f32))
                        ps = ppmm.tile([128, 512], f32, name="ps", tag="mm")
                        nc.tensor.matmul(
                            ps[0:64, :], qt_t[m][:], ktm[:],
                            start=True, stop=True)
                        e0 = smp2.tile([64, 512], f32, name="e0", tag="e0")
                        nc.vector.tensor_tensor(e0[:], ps[0:64, :],
                                                mt[0:64, :], ADD)
                        for j in range(4):
                            nc.scalar.activation(
                                ex_t[m][:, 128 * j:128 * (j + 1)],
                                e0[:, 128 * j:128 * (j + 1)], EXP,
                                accum_out=sums[:, 4 * m + j:4 * m + j + 1])

                    if stages < 4:
                        osb_stub = iop2.tile([128, H], f32, name="osb")
                        nc.vector.tensor_copy(osb_stub[:, 0:512], ex_t[0][:].broadcast_to([64, 512, 2]).rearrange("p a b -> p (a b)")[:, 0:512]) if False else None
                        nc.vector.tensor_copy(osb_stub[0:64, 0:512], ex_t[0][:])
                        nc.vector.tensor_copy(osb_stub[0:64, 512:1024], ex_t[7][:])
                        nc.vector.tensor_copy(osb_stub[64:128, :], osb_stub[0:64, :])
                        nc.sync.dma_start(
                            out_d[:, 4 * g:4 * (g + 1), :]
                            .rearrange("q s h -> s q h"), osb_stub[:])
                        return

                    # 4. V proj: v_j [128 st(t), 1024 hd] (fp32 out)
                    v_t = []
                    for j in range(4):
                        vj = iop.tile([128, H], f32, name=f"v{j}")
                        for n in range(2):
                            pv = ppmm.tile([128, 512], f32, name="pv",
                                           tag="mm")
                            for c in range(8):
                                nc.tensor.matmul(
                                    pv[:],
                                    xt_t[c][:, 128 * j:128 * (j + 1)],
                                    wv_t[c][:, 512 * n:512 * (n + 1)],
                                    start=(c == 0),
                                    stop=(c == 7 and not bias_v))
                            if bias_v:
                                nc.tensor.matmul(
                                    pv[:], ones_sb[:],
                                    bv_sb[:, 512 * n:512 * (n + 1)],
                                    start=False, stop=True)
                            nc.scalar.copy(vj[:, 512 * n:512 * (n + 1)],
                                           pv[:])
                        v_t.append(vj)

                    if stages < 5:
                        osb_stub = iop2.tile([128, H], f32, name="osb")
                        nc.vector.tensor_copy(osb_stub[:, 0:512], v_t[0][:, 0:512])
                        nc.vector.tensor_copy(osb_stub[:, 512:1024], v_t[3][:, 0:512])
                        nc.sync.dma_start(
                            out_d[:, 4 * g:4 * (g + 1), :]
                            .rearrange("q s h -> s q h"), osb_stub[:])
                        return

                    # normalize: alpha = ex * (1/rowsum)
                    recips = smp2.tile([64, 32], f32, name="recips")
                    nc.vector.reciprocal(recips[:], sums[:])
                    for m in range(8):
                        nc.vector.tensor_tensor(
                            ex_t[m].rearrange("p (s t) -> p s t", t=128),
                            ex_t[m].rearrange("p (s t) -> p s t", t=128),
                            recips[:, 4 * m:4 * (m + 1)]
                            .broadcast_to([64, 4, 128]),
                            MULT)

                    if debug_taps and g == 0:
                        nc.sync.dma_start(dbg["ex0"][:], ex_t[0][:])

                    # 8. alpha^T per pair: at_m [128 t, 4j x (2 x 32q)]
                    at_t = []
                    for m in range(8):
                        pat = ppat.tile([128, 256], f32, name="pat")
                        for j in range(4):
                            nc.tensor.transpose(
                                pat[:, 64 * j:64 * (j + 1)],
                                ex_t[m][:, 128 * j:128 * (j + 1)],
                                ident[0:64, 0:64])
                        atm = smp.tile([128, 256], f32, name="atm", tag="at",
                                       bufs=3)
                        nc.scalar.copy(atm[:], pat[:])
                        at_t.append(atm)
                        if debug_taps and g == 0 and m == 0:
                            nc.sync.dma_start(dbg["at0"][:], atm[:])

                    if stages < 6:
                        osb_stub = iop2.tile([128, H], f32, name="osb")
                        nc.vector.tensor_copy(osb_stub[:, 0:256], at_t[0][:])
                        nc.vector.tensor_copy(osb_stub[:, 256:512], at_t[7][:])
                        nc.vector.tensor_copy(osb_stub[:, 512:1024], osb_stub[:, 0:512])
                        nc.sync.dma_start(
                            out_d[:, 4 * g:4 * (g + 1), :]
                            .rearrange("q s h -> s q h"), osb_stub[:])
                        return

                    # 9. attn.V -> rT_half [128 hd-in-chunk, 4x(4s x 32q)]
                    rt_t = []
                    for half in range(2):
                        pr = ppr.tile([128, 512], f32, name="pr")
                        for cc in range(4):
                            c = 4 * half + cc
                            for h in (2 * c, 2 * c + 1):
                                ro = 64 * (h % 2)
                                for j in range(4):
                                    nc.tensor.matmul(
                                        pr[ro:ro + 64,
                                           128 * cc + 32 * j:
                                           128 * cc + 32 * (j + 1)],
                                        v_t[j][:, 64 * h:64 * (h + 1)],
                                        at_t[c][:, 64 * j + 32 * (h % 2):
                                                64 * j + 32 * (h % 2) + 32],
                                        start=True, stop=True,
                                        tile_position=(0, ro))
                        rth = smp.tile([128, 512], mdt, name=f"rt{half}")
                        nc.vector.tensor_copy(rth[:], pr[:])
                        rt_t.append(rth)
                        if debug_taps and g == 0 and half == 0:
                            nc.sync.dma_start(dbg["rt0"][:],
                                              rth[:].bitcast(f32))

                    if stages < 7:
                        osb_stub = iop2.tile([128, H], f32, name="osb")
                        nc.vector.tensor_copy(osb_stub[:, 0:512], rt_t[0][:].bitcast(f32))
                        nc.vector.tensor_copy(osb_stub[:, 512:1024], rt_t[1][:].bitcast(f32))
                        nc.sync.dma_start(
                            out_d[:, 4 * g:4 * (g + 1), :]
                            .rearrange("q s h -> s q h"), osb_stub[:])
                        return

                    # 10. O-proj: rows (4s x 32q) on partitions, H on free
                    osb = iop2.tile([128, H], f32, name="osb")
                    for n in range(2):
                        po = ppmm.tile([128, 512], f32, name="po", tag="mm")
                        for c in range(8):
                            nc.tensor.matmul(
                                po[:],
                                rt_t[c // 4][:, 128 * (c % 4):
                                             128 * (c % 4 + 1)],
                                wo_t[c][:, 512 * n:512 * (n + 1)],
                                start=(c == 0),
                                stop=(c == 7 and not bias_o))
                        if bias_o:
                            nc.tensor.matmul(
                                po[:], ones_sb[:],
                                bo_sb[:, 512 * n:512 * (n + 1)],
                                start=False, stop=True)
                        nc.scalar.copy(osb[:, 512 * n:512 * (n + 1)], po[:])

                    if debug_taps and g == 0:
                        nc.sync.dma_start(dbg["osb0"][:], osb[:])

                    if stages < 8:
                        nc.sync.dma_start(
                            out_d[:, 4 * g:4 * (g + 1), :]
                            .rearrange("q s h -> s q h"), osb[:])
                        return

                    # 11. LayerNorm over H (in place on osb)
                    s1 = smp2.tile([128, 1], f32, name="s1")
                    nc.vector.tensor_reduce(s1[:], osb[:], axis=AXX, op=ADD)
                    mean = smp2.tile([128, 1], f32, name="mean")
                    nc.vector.tensor_scalar(mean[:], s1[:], 1.0 / H, None,
                                            MULT)
                    nc.vector.tensor_scalar(osb[:], osb[:], mean[:], None,
                                            SUB)
                    sq = iop.tile([128, H], f32, name="sq", tag="x0")
                    ssq = smp2.tile([128, 1], f32, name="ssq")
                    nc.scalar.activation(sq[:], osb[:], SQUARE,
                                         accum_out=ssq[:])
                    stdv = smp2.tile([128, 1], f32, name="stdv")
                    nc.scalar.activation(stdv[:], ssq[:], SQRT,
                                         bias=eps_sb[:], scale=1.0 / H)
                    rstd = smp2.tile([128, 1], f32, name="rstd")
                    nc.vector.reciprocal(rstd[:], stdv[:])
                    nc.vector.tensor_scalar(osb[:], osb[:], rstd[:], None,
                                            MULT)
                    if gamma_beta:
                        nc.vector.tensor_tensor(osb[:], osb[:], gam_sb[:],
                                                MULT)
                        nc.vector.tensor_tensor(osb[:], osb[:], bet_sb[:],
                                                ADD)

                    # 12. out[q, 4g:4g+4, :] <- rows (s-major, q)
                    nc.sync.dma_start(
                        out_d[:, 4 * g:4 * (g + 1), :]
                        .rearrange("q s h -> s q h"),
                        osb[:])

                def emit_all():
                    for g in range(NG):
                        emit_chunk(g)

                if loop > 1:
                    with tc.For_i(0, loop, 1):
                        emit_all()
                else:
                    emit_all()

    nc.compile()
    return nc


def _get(loop=1, mm_fast=True, bias_kq=False, bias_v=False, bias_o=False,
         gamma_beta=False, debug_taps=False, stages=9):
    key = (loop, mm_fast, bias_kq, bias_v, bias_o, gamma_beta, debug_taps,
           stages)
    if key not in _BUILD_CACHE:
        _BUILD_CACHE[key] = _build(mm_fast=mm_fast, bias_kq=bias_kq,
                                   bias_v=bias_v, bias_o=bias_o,
                                   gamma_beta=gamma_beta, loop=loop,
                                   debug_taps=debug_taps, stages=stages)
    return _BUILD_CACHE[key]


def _in_maps(ini_q, ini_k, mask, Wq, bq, Wk, bk, Wv, bv, Wo, bo, gamma, beta):
    f = np.float32
    wqt = np.ascontiguousarray(np.asarray(Wq).T.astype(f) * f(0.125))
    wkt = np.ascontiguousarray(np.asarray(Wk).T.astype(f))
    wvt = np.ascontiguousarray(np.asarray(Wv).T.astype(f))
    wot = np.ascontiguousarray(np.asarray(Wo).T.astype(f))
    bqr = np.ascontiguousarray(
        (np.asarray(bq).astype(f) * f(0.125)).reshape(8, 128))
    bkr = np.ascontiguousarray(np.asarray(bk).astype(f).reshape(8, 128))
    shared = dict(wqt=wqt, wkt=wkt, wvt=wvt, wot=wot, bqr=bqr, bkr=bkr,
                  bvr=np.asarray(bv).astype(f).reshape(1, H),
                  bor=np.asarray(bo).astype(f).reshape(1, H),
                  gam=np.asarray(gamma).astype(f).reshape(1, H),
                  bet=np.asarray(beta).astype(f).reshape(1, H),
                  zpad=np.zeros((128, 64), f))
    ini_q = np.asarray(ini_q)
    ini_k = np.asarray(ini_k)
    mask = np.asarray(mask)
    maps = []
    for b in range(B):
        m = dict(shared)
        m["xq"] = np.ascontiguousarray(ini_q[b].astype(f))
        m["xk"] = np.ascontiguousarray(ini_k[b].astype(f).reshape(ST, H))
        m["mneg"] = np.ascontiguousarray(mask[b].astype(f) * f(-10000.0))
        maps.append(m)
    return maps


def run(inputs, loop=1, mm_fast=True, debug_taps=False, full_results=False,
        stages=9):
    """Run the SPMD kernel; returns (B, Q, S, H) float32."""
    from concourse.bass_utils import run_bass_kernel_spmd

    flags = dict(
        debug_taps=debug_taps, stages=stages,
        bias_kq=bool(np.any(inputs["bq"]) or np.any(inputs["bk"])),
        bias_v=bool(np.any(inputs["bv"])),
        bias_o=bool(np.any(inputs["bo"])),
        gamma_beta=bool(np.any(np.asarray(inputs["gamma"]) != 1.0)
                        or np.any(inputs["beta"])),
    )
    nc = _get(loop=loop, mm_fast=mm_fast, **flags)
    maps = _in_maps(**inputs)
    err = None
    for _ in range(4):
        try:
            res = run_bass_kernel_spmd(nc, maps, list(range(NCORES)))
            break
        except Exception as e:  # transient NRT device errors: retry
            err = e
            import time as _t
            _t.sleep(2.0)
    else:
        raise err
    if full_results:
        return res
    return np.stack([res.results[c]["out"] for c in range(NCORES)], axis=0)


def kernel(**inputs):
    return run(inputs, loop=1, mm_fast=True)



# revision 2
# speedup vs baseline: 3.6201x; 3.6201x over previous
"""Trainium2 Bass kernel for nn_MultiHeadTokenAttention — v2.

Reference computation (per batch element b):
    q = ini_q @ Wq.T + bq                      [Q, H] -> heads [Q, 16, 64]
    k = X @ Wk.T + bk ;  v = X @ Wv.T + bv     (X = ini_k[b] as [S*T, H])
    scores[h,q,s,t] = (q_h . k_h) / 8,  + mask*-1e4, softmax over t
    res[q,s,:] = concat_h(sum_t alpha * v_h)   [Q, S, H]
    res = res @ Wo.T + bo;  LayerNorm(res) * gamma + beta

Sharding: batch-parallel, one batch element per NeuronCore (8 cores, no
collectives).

v2 structure (host + device):
  * Host folds Wk into the queries:  qk[32h+q, :] = scale * q_h @ Wk_h
    so  scoresT[st, hq] = X @ qk^T  -- the K projection is never
    materialized (4.3G MACs instead of 8.9G) and no K^T is needed.
  * Host pre-transposes X to bf16 X^T, tiled [16 chunks][128 p][8 c][512]
    so each chunk's load is one fully-contiguous 1 MB DMA and the device
    does zero transposes (PE transposes were ~20% of baseline PE time).
  * Softmax runs in the transposed layout [t on partitions, hq free]:
    mask enters as the per-partition bias of the exp activation; column
    sums via a ones-stationary matmul; 1/Z broadcast across partitions
    via a K=1 matmul; one DVE multiply -> alphaT (bf16).
  * attn.V: lhsT = V_s [128 t, 128 hd (2 heads)], rhs = alphaT cols of
    the same 2 heads -> out [128 hd, 64]; diagonal 64x32 blocks are the
    valid res^T entries, gathered by 2 strided DVE copies per head-pair
    into rt_c [128 hd, 128 (s,q)] which feeds the O projection as its
    stationary operand unchanged.  LayerNorm as in v1.

All matmuls run bf16 (f32 PSUM accumulate); rel err vs f32 reference is
~2e-3, comfortably under the 2e-2 gate.
"""

import os
import sys

for _p in ("/opt/trn_rl_repo", "/root/.axon_site/_ro/trn_rl_repo"):
    if os.path.isdir(_p) and _p not in sys.path:
        sys.path.insert(0, _p)

import numpy as np

B, Q, S, T, H = 8, 32, 64, 128, 1024
HEADS, D = 16, 64
ST = S * T           # 8192 rows of X per batch element
NCORES = 8
NG = 16              # chunks per core (4 s-values = 512 st rows each)
HQ = HEADS * Q       # 512
EPS = 1e-12

_BUILD_CACHE = {}

# softmax 1/Z plumbing: "mm" = colsum+broadcast via PE matmuls;
# "ar" = gpsimd partition_all_reduce + DVE recip/mult (no PE work, no PSUM)
ZMODE = os.environ.get("KV2_ZMODE", "ar")


def _build(bias_kq=False, bias_v=False, bias_o=False, gamma_beta=False,
           loop=1, stages=9):
    """Build + compile the Bass program. Returns the Bacc object."""
    import concourse.mybir as mybir
    from concourse import bacc
    from concourse.tile import TileContext

    f32 = mybir.dt.float32
    bf16 = mybir.dt.bfloat16
    ADD = mybir.AluOpType.add
    SUB = mybir.AluOpType.subtract
    MULT = mybir.AluOpType.mult
    AXX = mybir.AxisListType.X
    EXP = mybir.ActivationFunctionType.Exp
    LN_F = mybir.ActivationFunctionType.Ln
    DIV = mybir.AluOpType.divide
    from concourse import bass_isa

    nc = bacc.Bacc("TRN2", target_bir_lowering=False, debug=False,
                   num_devices=NCORES)

    # X^T bf16, tiled: xkt[g, p, c, j] = X[512 g + j, 128 c + p]
    xkt_d = nc.dram_tensor("xkt", [NG, 128, 8, 512], bf16,
                           kind="ExternalInput")
    # qk^T bf16: qkt[c, p, m] = qk[m, 128 c + p]  (m = 32 h + q)
    qkt_d = nc.dram_tensor("qkt", [8, 128, HQ], bf16, kind="ExternalInput")
    # mask^T * -1e4: mnegt[t, s]
    mnegt_d = nc.dram_tensor("mnegt", [T, S], f32, kind="ExternalInput")
    # Wv^T bf16 rows h cols hd; Wo^T bf16 rows hd cols H
    wvt_d = nc.dram_tensor("wvt", [H, H], bf16, kind="ExternalInput")
    wot_d = nc.dram_tensor("wot", [H, H], bf16, kind="ExternalInput")
    bkq_d = nc.dram_tensor("bkq", [1, HQ], bf16, kind="ExternalInput")
    bv_d = nc.dram_tensor("bvr", [1, H], bf16, kind="ExternalInput")
    bo_d = nc.dram_tensor("bor", [1, H], bf16, kind="ExternalInput")
    gam_d = nc.dram_tensor("gam", [1, H], f32, kind="ExternalInput")
    bet_d = nc.dram_tensor("bet", [1, H], f32, kind="ExternalInput")
    # s-major output: contiguous 512 KB write per chunk (the q-major layout
    # costs 128 scattered 4 KB descriptors per chunk and dominates the
    # critical path); host returns a transposed view.
    out_d = nc.dram_tensor("out", [S, Q, H], f32, kind="ExternalOutput")

    with TileContext(nc) as tc:
        with tc.tile_pool(name="wts", bufs=1) as wpool, \
             tc.tile_pool(name="ppmm", bufs=3, space="PSUM") as ppmm, \
             tc.tile_pool(name="ppz", bufs=1, space="PSUM") as ppz, \
             tc.tile_pool(name="ppzb", bufs=2, space="PSUM") as ppzb, \
             tc.tile_pool(name="ppav", bufs=2, space="PSUM") as ppav:

            # ---------------- preamble: constants + weights ----------------
            eps_sb = wpool.tile([128, 1], f32, name="eps_sb")
            nc.vector.memset(eps_sb[:], EPS)
            ones_col = wpool.tile([128, 1], bf16, name="ones_col")
            nc.vector.memset(ones_col[:], 1.0)
            ones_row = wpool.tile([1, 128], bf16, name="ones_row")
            nc.vector.memset(ones_row[:], 1.0)
            ones_row_f = wpool.tile([1, 128], f32, name="ones_row_f")
            nc.vector.memset(ones_row_f[:], 1.0)

            mneg_sb = wpool.tile([T, S], f32, name="mneg_sb")
            nc.sync.dma_start(mneg_sb[:], mnegt_d[:])

            qk_sb, wv_sb, wo_sb = [], [], []
            for c in range(8):
                qkc = wpool.tile([128, HQ], bf16, name=f"qk{c}")
                nc.gpsimd.dma_start(qkc[:], qkt_d[c])
                qk_sb.append(qkc)
                wvc = wpool.tile([128, H], bf16, name=f"wv{c}")
                nc.gpsimd.dma_start(wvc[:], wvt_d[128 * c:128 * (c + 1), :])
                wv_sb.append(wvc)
                woc = wpool.tile([128, H], bf16, name=f"wo{c}")
                nc.gpsimd.dma_start(woc[:], wot_d[128 * c:128 * (c + 1), :])
                wo_sb.append(woc)

            if bias_kq:
                bkq_sb = wpool.tile([1, HQ], bf16, name="bkq_sb")
                nc.gpsimd.dma_start(bkq_sb[:], bkq_d[:])
            if bias_v:
                bv_sb = wpool.tile([1, H], bf16, name="bv_sb")
                nc.gpsimd.dma_start(bv_sb[:], bv_d[:])
            if bias_o:
                bo_sb = wpool.tile([1, H], bf16, name="bo_sb")
                nc.gpsimd.dma_start(bo_sb[:], bo_d[:])
            if gamma_beta:
                gam_sb = wpool.tile([128, H], f32, name="gam_sb")
                bet_sb = wpool.tile([128, H], f32, name="bet_sb")
                nc.sync.dma_start(
                    gam_sb[:], gam_d[0, :].partition_broadcast(128))
                nc.sync.dma_start(
                    bet_sb[:], bet_d[0, :].partition_broadcast(128))

            # ---------------- main per-chunk pipeline ----------------
            with tc.tile_pool(name="io", bufs=3) as iop, \
                 tc.tile_pool(name="sm", bufs=2) as smp, \
                 tc.tile_pool(name="ln", bufs=2) as lnp:

                def emit_chunk(g):
                    # 1. load X^T chunk: one contiguous 1 MB DMA.  Issued on
                    # the (otherwise idle) Pool queue so it never queues
                    # behind the out-store on SP.
                    xt = iop.tile([128, 4096], bf16, name="xt")
                    nc.sync.dma_start(
                        xt[:].rearrange("p (c j) -> p c j", c=8), xkt_d[g])

                    ex_t, al_t, v_t = [], [], []
                    for sp in range(4):
                        # 2. scoresT[t, hq] for s = 4g+sp
                        ps = ppmm.tile([128, 512], f32, name="ps", tag="mm")
                        for c in range(8):
                            nc.tensor.matmul(
                                ps[:], xt[:, 512 * c + 128 * sp:
                                          512 * c + 128 * (sp + 1)],
                                qk_sb[c][:],
                                start=(c == 0),
                                stop=(c == 7 and not bias_kq))
                        if bias_kq:
                            nc.tensor.matmul(ps[:], ones_row[:], bkq_sb[:],
                                             start=False, stop=True)
                        # 3. exp(scoresT + mask_col) -> bf16, mask via bias
                        ex = smp.tile([128, 512], bf16, name=f"ex{sp}")
                        nc.scalar.activation(
                            ex[:], ps[:], EXP,
                            bias=mneg_sb[:, 4 * g + sp:4 * g + sp + 1])
                        ex_t.append(ex)
                        al = smp.tile([128, 512], bf16, name=f"al{sp}")
                        if ZMODE == "ar":
                            # 4+5. Z bcast via gpsimd all-reduce;
                            # al = ex * (1/Z)  (DVE divide is not valid ISA)
                            zsb = smp.tile([128, 512], f32, name="zsb",
                                           tag="zsb", bufs=2)
                            nc.gpsimd.partition_all_reduce(
                                zsb[:], ex[:], 128, bass_isa.ReduceOp.add)
                            zrb = smp.tile([128, 512], f32, name="zrb",
                                           tag="zrb", bufs=2)
                            nc.vector.reciprocal(zrb[:], zsb[:])
                            nc.vector.tensor_tensor(al[:], ex[:], zrb[:],
                                                    MULT)
                        else:
                            # 4. Z[hq] colsums via ones-stationary matmul
                            z = ppz.tile([1, 512], f32, name="z", tag="z")
                            nc.tensor.matmul(z[:], ones_col[:], ex[:],
                                             start=True, stop=True)
                            zr = smp.tile([1, 512], f32, name="zr",
                                          tag="zr", bufs=2)
                            nc.vector.reciprocal(zr[:], z[:])
                            # 5. bcast 1/Z across partitions via K=1 matmul
                            zb = ppzb.tile([128, 512], f32, name="zb",
                                           tag="zb")
                            nc.tensor.matmul(zb[:], ones_row_f[:], zr[:],
                                             start=True, stop=True)
                            nc.vector.tensor_tensor(al[:], ex[:], zb[:],
                                                    MULT)
                        al_t.append(al)
                        # 6. V_s[t, hd] natural
                        vs = smp.tile([128, H], bf16, name=f"v{sp}")
                        for n in range(2):
                            pv = ppmm.tile([128, 512], f32, name="pv",
                                           tag="mm")
                            for c in range(8):
                                nc.tensor.matmul(
                                    pv[:],
                                    xt[:, 512 * c + 128 * sp:
                                       512 * c + 128 * (sp + 1)],
                                    wv_sb[c][:, 512 * n:512 * (n + 1)],
                                    start=(c == 0),
                                    stop=(c == 7 and not bias_v))
                            if bias_v:
                                nc.tensor.matmul(
                                    pv[:], ones_row[:],
                                    bv_sb[:, 512 * n:512 * (n + 1)],
                                    start=False, stop=True)
                            nc.scalar.copy(vs[:, 512 * n:512 * (n + 1)],
                                           pv[:])
                        v_t.append(vs)

                    if stages < 4:
                        osb_stub = lnp.tile([128, H], f32, name="osb")
                        nc.vector.tensor_copy(osb_stub[:, 0:512], al_t[0][:])
                        nc.vector.tensor_copy(osb_stub[:, 512:1024],
                                              v_t[3][:, 0:512])
                        nc.sync.dma_start(out_d[4 * g:4 * (g + 1)],
                                          osb_stub[:])
                        return

                    # 7. attn.V -> rt_c [128 hd (2 heads), 128 (s, q)]
                    # pav is a full PSUM bank: a half-bank tile would share
                    # its physical bank with the pool's other rotation buf,
                    # and PE-write + DVE-read of one bank is a fatal HW
                    # PSUM collision (not modeled by CoreSim).
                    rt_t = []
                    for c in range(8):
                        pav = ppav.tile([128, 512], f32, name="pav",
                                        tag="av")
                        for sp in range(4):
                            nc.tensor.matmul(
                                pav[:, 64 * sp:64 * (sp + 1)],
                                v_t[sp][:, 128 * c:128 * (c + 1)],
                                al_t[sp][:, 64 * c:64 * (c + 1)],
                                start=True, stop=True)
                        rt = smp.tile([128, 128], bf16, name=f"rt{c}")
                        nc.vector.tensor_copy(
                            rt[0:64, :].rearrange("p (s q) -> p s q", q=32),
                            pav[0:64, 0:256]
                            .rearrange("p (s q2) -> p s q2", q2=64)[:, :, 0:32])
                        nc.vector.tensor_copy(
                            rt[64:128, :].rearrange("p (s q) -> p s q", q=32),
                            pav[64:128, 0:256]
                            .rearrange("p (s q2) -> p s q2", q2=64)[:, :, 32:64])
                        rt_t.append(rt)

                    if stages < 6:
                        osb_stub = lnp.tile([128, H], f32, name="osb")
                        nc.vector.tensor_copy(osb_stub[:, 0:64],
                                              rt_t[0][:].bitcast(f32))
                        nc.vector.tensor_copy(osb_stub[:, 64:128],
                                              rt_t[7][:].bitcast(f32))
                        nc.vector.tensor_copy(osb_stub[:, 128:256],
                                              osb_stub[:, 0:128])
                        nc.vector.tensor_copy(osb_stub[:, 256:512],
                                              osb_stub[:, 0:256])
                        nc.vector.tensor_copy(osb_stub[:, 512:1024],
                                              osb_stub[:, 0:512])
                        nc.sync.dma_start(out_d[4 * g:4 * (g + 1)],
                                          osb_stub[:])
                        return

                    # 8. O-proj: rows (s, q) on partitions, H on free
                    osb = lnp.tile([128, H], f32, name="osb")
                    for n in range(2):
                        po = ppmm.tile([128, 512], f32, name="po", tag="mm")
                        for c in range(8):
                            nc.tensor.matmul(
                                po[:], rt_t[c][:],
                                wo_sb[c][:, 512 * n:512 * (n + 1)],
                                start=(c == 0),
                                stop=(c == 7 and not bias_o))
                        if bias_o:
                            nc.tensor.matmul(
                                po[:], ones_row[:],
                                bo_sb[:, 512 * n:512 * (n + 1)],
                                start=False, stop=True)
                        nc.scalar.copy(osb[:, 512 * n:512 * (n + 1)], po[:])

                    # 9. LayerNorm over H (in place on osb).  rstd via the
                    # DVE pow ALU op, so ACT only ever needs exp+copy (one
                    # act-table load, hoisted out of the loop).
                    s1 = lnp.tile([128, 1], f32, name="s1")
                    nc.vector.tensor_reduce(s1[:], osb[:], axis=AXX, op=ADD)
                    mean = lnp.tile([128, 1], f32, name="mean")
                    nc.vector.tensor_scalar(mean[:], s1[:], 1.0 / H, None,
                                            MULT)
                    nc.vector.tensor_scalar(osb[:], osb[:], mean[:], None,
                                            SUB)
                    sq = lnp.tile([128, H], f32, name="sq")
                    nc.vector.tensor_tensor(sq[:], osb[:], osb[:], MULT)
                    ssq = lnp.tile([128, 1], f32, name="ssq")
                    nc.vector.tensor_reduce(ssq[:], sq[:], axis=AXX, op=ADD)
                    # ln(ssq/H + eps) via the activation's scale+bias, then
                    # rstd = exp(-0.5 ln(var+eps)); both funcs live in the
                    # pinned act table set.
                    lnv = lnp.tile([128, 1], f32, name="lnv")
                    nc.scalar.activation(lnv[:], ssq[:], LN_F,
                                         bias=eps_sb[:], scale=1.0 / H)
                    rstd = lnp.tile([128, 1], f32, name="rstd")
                    nc.scalar.activation(rstd[:], lnv[:], EXP, scale=-0.5)
                    nc.vector.tensor_scalar(osb[:], osb[:], rstd[:], None,
                                            MULT)
                    if gamma_beta:
                        nc.vector.tensor_tensor(osb[:], osb[:], gam_sb[:],
                                                MULT)
                        nc.vector.tensor_tensor(osb[:], osb[:], bet_sb[:],
                                                ADD)

                    # 10. out[4g:4g+4, :, :] <- rows (s-major, q); fully
                    # contiguous 512 KB store.
                    nc.sync.dma_start(out_d[4 * g:4 * (g + 1)], osb[:])

                def emit_all():
                    for g in range(NG):
                        emit_chunk(g)

                if loop > 1:
                    with tc.For_i(0, loop, 1):
                        emit_all()
                else:
                    emit_all()

    # Pin exp/ln/copy activations to the one act-table set that holds all
    # of them ("natural_log_exp_and_others") so the table load is emitted
    # once and hoisted out of the loop instead of swapping every chunk.
    import concourse.bacc as bacc_mod
    _orig_gat = bacc_mod.get_activation_tables
    _pin = {mybir.ActivationFunctionType.Exp, mybir.ActivationFunctionType.Ln,
            mybir.ActivationFunctionType.Copy,
            mybir.ActivationFunctionType.Identity}

    def _gat(arch):
        tables = _orig_gat(arch)
        return {name: (funcs if name == "natural_log_exp_and_others"
                       else funcs - _pin)
                for name, funcs in tables.items()}

    bacc_mod.get_activation_tables = _gat
    try:
        nc.compile()
    finally:
        bacc_mod.get_activation_tables = _orig_gat
    return nc


def _get(loop=1, bias_kq=False, bias_v=False, bias_o=False,
         gamma_beta=False, stages=9):
    key = (loop, bias_kq, bias_v, bias_o, gamma_beta, stages)
    if key not in _BUILD_CACHE:
        _BUILD_CACHE[key] = _build(bias_kq=bias_kq, bias_v=bias_v,
                                   bias_o=bias_o, gamma_beta=gamma_beta,
                                   loop=loop, stages=stages)
    return _BUILD_CACHE[key]


_PREP_CACHE = {}


def _prep_fns():
    """jitted CPU preprocessing (transpose/cast are multithreaded in XLA)."""
    if _PREP_CACHE:
        return _PREP_CACHE
    import jax
    import jax.numpy as jnp

    cpu = jax.devices("cpu")[0]

    def _xkt(x):  # [S*T, H] f32 -> [16, 128, 8, 512] bf16
        x4 = x.reshape(NG, 512, 8, 128)
        return x4.transpose(0, 3, 2, 1).astype(jnp.bfloat16)

    def _qkt(ini_q, Wq, bq, Wk):  # -> [8, 128, HQ] bf16
        q = ini_q @ Wq.T + bq                      # [Q, H]
        qh = q.reshape(Q, HEADS, D)
        qk = jnp.einsum("qhd,hdH->hqH", qh,
                        Wk.reshape(HEADS, D, H)) * np.float32(0.125)
        qkt = qk.reshape(HQ, H).T                  # [H, HQ]
        return qkt.reshape(8, 128, HQ).astype(jnp.bfloat16)

    def _bkq(ini_q, Wq, bq, bk):  # -> [1, HQ] bf16
        q = ini_q @ Wq.T + bq
        qh = q.reshape(Q, HEADS, D)
        t2 = jnp.einsum("qhd,hd->hq", qh,
                        bk.reshape(HEADS, D)) * np.float32(0.125)
        return t2.reshape(1, HQ).astype(jnp.bfloat16)

    _PREP_CACHE["xkt"] = jax.jit(_xkt, device=cpu)
    _PREP_CACHE["qkt"] = jax.jit(_qkt, device=cpu)
    _PREP_CACHE["bkq"] = jax.jit(_bkq, device=cpu)
    return _PREP_CACHE


def _in_maps(ini_q, ini_k, mask, Wq, bq, Wk, bk, Wv, bv, Wo, bo, gamma, beta):
    import ml_dtypes
    f = np.float32
    bfdt = ml_dtypes.bfloat16
    fns = _prep_fns()

    wvt = np.asarray(Wv, dtype=f).T.astype(bfdt)
    wot = np.asarray(Wo, dtype=f).T.astype(bfdt)
    shared = dict(
        wvt=np.ascontiguousarray(wvt),
        wot=np.ascontiguousarray(wot),
        bvr=np.asarray(bv, dtype=f).reshape(1, H).astype(bfdt),
        bor=np.asarray(bo, dtype=f).reshape(1, H).astype(bfdt),
        gam=np.asarray(gamma, dtype=f).reshape(1, H),
        bet=np.asarray(beta, dtype=f).reshape(1, H),
    )
    ini_q = np.asarray(ini_q, dtype=f)
    ini_k = np.asarray(ini_k, dtype=f)
    mask = np.asarray(mask, dtype=f)
    Wq_, bq_, Wk_, bk_ = (np.asarray(a, dtype=f) for a in (Wq, bq, Wk, bk))
    maps = []
    for b in range(B):
        m = dict(shared)
        m["xkt"] = np.asarray(fns["xkt"](ini_k[b].reshape(ST, H)))
        m["qkt"] = np.asarray(fns["qkt"](ini_q[b], Wq_, bq_, Wk_))
        m["bkq"] = np.asarray(fns["bkq"](ini_q[b], Wq_, bq_, bk_))
        m["mnegt"] = np.ascontiguousarray(mask[b].T * f(-10000.0))
        maps.append(m)
    return maps


def run(inputs, loop=1, full_results=False, stages=9):
    """Run the SPMD kernel; returns (B, Q, S, H) float32."""
    from concourse.bass_utils import run_bass_kernel_spmd

    flags = dict(
        stages=stages,
        bias_kq=bool(np.any(inputs["bq"]) or np.any(inputs["bk"])),
        bias_v=bool(np.any(inputs["bv"])),
        bias_o=bool(np.any(inputs["bo"])),
        gamma_beta=bool(np.any(np.asarray(inputs["gamma"]) != 1.0)
                        or np.any(inputs["beta"])),
    )
    nc = _get(loop=loop, **flags)
    maps = _in_maps(**inputs)
    err = None
    for _ in range(4):
        try:
            res = run_bass_kernel_spmd(nc, maps, list(range(NCORES)))
            break
        except Exception as e:  # transient NRT device errors: retry
            err = e
            import time as _t
            _t.sleep(2.0)
    else:
        raise err
    if full_results:
        return res
    # device output is s-major [S, Q, H]; transpose back to [Q, S, H]
    return np.stack([res.results[c]["out"].transpose(1, 0, 2)
                     for c in range(NCORES)], axis=0)


def kernel(**inputs):
    return run(inputs, loop=1)


# revision 4
# speedup vs baseline: 34.9458x; 9.6533x over previous
"""Trainium2 Bass kernel for nn_MultiHeadTokenAttention — v2.

Reference computation (per batch element b):
    q = ini_q @ Wq.T + bq                      [Q, H] -> heads [Q, 16, 64]
    k = X @ Wk.T + bk ;  v = X @ Wv.T + bv     (X = ini_k[b] as [S*T, H])
    scores[h,q,s,t] = (q_h . k_h) / 8,  + mask*-1e4, softmax over t
    res[q,s,:] = concat_h(sum_t alpha * v_h)   [Q, S, H]
    res = res @ Wo.T + bo;  LayerNorm(res) * gamma + beta

Sharding: batch-parallel, one batch element per NeuronCore (8 cores, no
collectives).

v2 structure (host + device):
  * Host folds Wk into the queries:  qk[32h+q, :] = scale * q_h @ Wk_h
    so  scoresT[st, hq] = X @ qk^T  -- the K projection is never
    materialized (4.3G MACs instead of 8.9G) and no K^T is needed.
  * Host pre-transposes X to bf16 X^T, tiled [16 chunks][128 p][8 c][512]
    so each chunk's load is one fully-contiguous 1 MB DMA and the device
    does zero transposes (PE transposes were ~20% of baseline PE time).
  * Softmax runs in the transposed layout [t on partitions, hq free]:
    mask enters as the per-partition bias of the exp activation; column
    sums via a ones-stationary matmul; 1/Z broadcast across partitions
    via a K=1 matmul; one DVE multiply -> alphaT (bf16).
  * attn.V: lhsT = V_s [128 t, 128 hd (2 heads)], rhs = alphaT cols of
    the same 2 heads -> out [128 hd, 64]; diagonal 64x32 blocks are the
    valid res^T entries, gathered by 2 strided DVE copies per head-pair
    into rt_c [128 hd, 128 (s,q)] which feeds the O projection as its
    stationary operand unchanged.  LayerNorm as in v1.

All matmuls run bf16 (f32 PSUM accumulate); rel err vs f32 reference is
~2e-3, comfortably under the 2e-2 gate.
"""

import os
import sys

for _p in ("/opt/trn_rl_repo", "/root/.axon_site/_ro/trn_rl_repo"):
    if os.path.isdir(_p) and _p not in sys.path:
        sys.path.insert(0, _p)

import numpy as np

B, Q, S, T, H = 8, 32, 64, 128, 1024
HEADS, D = 16, 64
ST = S * T           # 8192 rows of X per batch element
NCORES = 8
NG = 16              # chunks per core (4 s-values = 512 st rows each)
HQ = HEADS * Q       # 512
EPS = 1e-12

_BUILD_CACHE = {}

# softmax 1/Z plumbing: "mm" = colsum+broadcast via PE matmuls;
# "ar" = gpsimd partition_all_reduce + DVE recip/mult (no PE work, no PSUM)
ZMODE = os.environ.get("KV2_ZMODE", "ar")


def _build(bias_kq=False, bias_v=False, bias_o=False, gamma_beta=False,
           loop=1, stages=9):
    """Build + compile the Bass program. Returns the Bacc object."""
    import concourse.mybir as mybir
    from concourse import bacc
    from concourse.tile import TileContext

    f32 = mybir.dt.float32
    bf16 = mybir.dt.bfloat16
    ADD = mybir.AluOpType.add
    SUB = mybir.AluOpType.subtract
    MULT = mybir.AluOpType.mult
    AXX = mybir.AxisListType.X
    EXP = mybir.ActivationFunctionType.Exp
    LN_F = mybir.ActivationFunctionType.Ln
    DIV = mybir.AluOpType.divide
    from concourse import bass_isa

    nc = bacc.Bacc("TRN2", target_bir_lowering=False, debug=False,
                   num_devices=NCORES)

    # X^T bf16, tiled: xkt[g, p, c, j] = X[512 g + j, 128 c + p]
    xkt_d = nc.dram_tensor("xkt", [NG, 128, 8, 512], bf16,
                           kind="ExternalInput")
    # qk^T bf16: qkt[c, p, m] = qk[m, 128 c + p]  (m = 32 h + q)
    qkt_d = nc.dram_tensor("qkt", [8, 128, HQ], bf16, kind="ExternalInput")
    # mask^T * -1e4: mnegt[t, s]
    mnegt_d = nc.dram_tensor("mnegt", [T, S], f32, kind="ExternalInput")
    # Wv^T bf16 rows h cols hd; Wo^T bf16 rows hd cols H
    wvt_d = nc.dram_tensor("wvt", [H, H], bf16, kind="ExternalInput")
    wot_d = nc.dram_tensor("wot", [H, H], bf16, kind="ExternalInput")
    bkq_d = nc.dram_tensor("bkq", [1, HQ], bf16, kind="ExternalInput")
    bv_d = nc.dram_tensor("bvr", [1, H], bf16, kind="ExternalInput")
    bo_d = nc.dram_tensor("bor", [1, H], bf16, kind="ExternalInput")
    gam_d = nc.dram_tensor("gam", [1, H], f32, kind="ExternalInput")
    bet_d = nc.dram_tensor("bet", [1, H], f32, kind="ExternalInput")
    # s-major output: contiguous 512 KB write per chunk (the q-major layout
    # costs 128 scattered 4 KB descriptors per chunk and dominates the
    # critical path); host returns a transposed view.
    out_d = nc.dram_tensor("out", [S, Q, H], f32, kind="ExternalOutput")

    mm_bufs = 3 if ZMODE == "mm" else 5
    with TileContext(nc) as tc:
        with tc.tile_pool(name="wts", bufs=1) as wpool, \
             tc.tile_pool(name="ppmm", bufs=mm_bufs, space="PSUM") as ppmm, \
             tc.tile_pool(name="ppz", bufs=1, space="PSUM") as ppz, \
             tc.tile_pool(name="ppzb", bufs=2, space="PSUM") as ppzb, \
             tc.tile_pool(name="ppav", bufs=2, space="PSUM") as ppav:

            # ---------------- preamble: constants + weights ----------------
            eps_sb = wpool.tile([128, 1], f32, name="eps_sb")
            nc.vector.memset(eps_sb[:], EPS)
            ones_col = wpool.tile([128, 1], bf16, name="ones_col")
            nc.vector.memset(ones_col[:], 1.0)
            ones_row = wpool.tile([1, 128], bf16, name="ones_row")
            nc.vector.memset(ones_row[:], 1.0)
            ones_row_f = wpool.tile([1, 128], f32, name="ones_row_f")
            nc.vector.memset(ones_row_f[:], 1.0)

            mneg_sb = wpool.tile([T, S], f32, name="mneg_sb")
            nc.sync.dma_start(mneg_sb[:], mnegt_d[:])

            qk_sb, wv_sb, wo_sb = [], [], []
            for c in range(8):
                qkc = wpool.tile([128, HQ], bf16, name=f"qk{c}")
                nc.gpsimd.dma_start(qkc[:], qkt_d[c])
                qk_sb.append(qkc)
                wvc = wpool.tile([128, H], bf16, name=f"wv{c}")
                nc.gpsimd.dma_start(wvc[:], wvt_d[128 * c:128 * (c + 1), :])
                wv_sb.append(wvc)
                woc = wpool.tile([128, H], bf16, name=f"wo{c}")
                nc.gpsimd.dma_start(woc[:], wot_d[128 * c:128 * (c + 1), :])
                wo_sb.append(woc)

            if bias_kq:
                bkq_sb = wpool.tile([1, HQ], bf16, name="bkq_sb")
                nc.gpsimd.dma_start(bkq_sb[:], bkq_d[:])
            if bias_v:
                bv_sb = wpool.tile([1, H], bf16, name="bv_sb")
                nc.gpsimd.dma_start(bv_sb[:], bv_d[:])
            if bias_o:
                bo_sb = wpool.tile([1, H], bf16, name="bo_sb")
                nc.gpsimd.dma_start(bo_sb[:], bo_d[:])
            if gamma_beta:
                gam_sb = wpool.tile([128, H], f32, name="gam_sb")
                bet_sb = wpool.tile([128, H], f32, name="bet_sb")
                nc.sync.dma_start(
                    gam_sb[:], gam_d[0, :].partition_broadcast(128))
                nc.sync.dma_start(
                    bet_sb[:], bet_d[0, :].partition_broadcast(128))

            # ---------------- main per-chunk pipeline ----------------
            with tc.tile_pool(name="io", bufs=3) as iop, \
                 tc.tile_pool(name="sm", bufs=2) as smp, \
                 tc.tile_pool(name="ln", bufs=2) as lnp:

                def emit_chunk(g):
                    # 1. load X^T chunk: one contiguous 1 MB DMA.  Issued on
                    # the (otherwise idle) Pool queue so it never queues
                    # behind the out-store on SP.
                    xt = iop.tile([128, 4096], bf16, name="xt")
                    nc.sync.dma_start(
                        xt[:].rearrange("p (c j) -> p c j", c=8), xkt_d[g])

                    ex_t, al_t, v_t = [], [], []
                    for sp in range(4):
                        # 2. scoresT[t, hq] for s = 4g+sp
                        ps = ppmm.tile([128, 512], f32, name="ps", tag="mm")
                        for c in range(8):
                            nc.tensor.matmul(
                                ps[:], xt[:, 512 * c + 128 * sp:
                                          512 * c + 128 * (sp + 1)],
                                qk_sb[c][:],
                                start=(c == 0),
                                stop=(c == 7 and not bias_kq))
                        if bias_kq:
                            nc.tensor.matmul(ps[:], ones_row[:], bkq_sb[:],
                                             start=False, stop=True)
                        # 3. exp(scoresT + mask_col) -> bf16, mask via bias
                        ex = smp.tile([128, 512], bf16, name=f"ex{sp}")
                        nc.scalar.activation(
                            ex[:], ps[:], EXP,
                            bias=mneg_sb[:, 4 * g + sp:4 * g + sp + 1])
                        ex_t.append(ex)
                        al = smp.tile([128, 512], bf16, name=f"al{sp}")
                        if ZMODE == "ar":
                            # 4+5. Z bcast via gpsimd all-reduce;
                            # al = ex * (1/Z)  (DVE divide is not valid ISA)
                            zsb = smp.tile([128, 512], f32, name="zsb",
                                           tag="zsb", bufs=2)
                            nc.gpsimd.partition_all_reduce(
                                zsb[:], ex[:], 128, bass_isa.ReduceOp.add)
                            zrb = smp.tile([128, 512], f32, name="zrb",
                                           tag="zrb", bufs=2)
                            nc.vector.reciprocal(zrb[:], zsb[:])
                            nc.vector.tensor_tensor(al[:], ex[:], zrb[:],
                                                    MULT)
                        else:
                            # 4. Z[hq] colsums via ones-stationary matmul
                            z = ppz.tile([1, 512], f32, name="z", tag="z")
                            nc.tensor.matmul(z[:], ones_col[:], ex[:],
                                             start=True, stop=True)
                            zr = smp.tile([1, 512], f32, name="zr",
                                          tag="zr", bufs=2)
                            nc.vector.reciprocal(zr[:], z[:])
                            # 5. bcast 1/Z across partitions via K=1 matmul
                            zb = ppzb.tile([128, 512], f32, name="zb",
                                           tag="zb")
                            nc.tensor.matmul(zb[:], ones_row_f[:], zr[:],
                                             start=True, stop=True)
                            nc.vector.tensor_tensor(al[:], ex[:], zb[:],
                                                    MULT)
                        al_t.append(al)
                        # 6. V_s[t, hd] natural
                        vs = smp.tile([128, H], bf16, name=f"v{sp}")
                        for n in range(2):
                            pv = ppmm.tile([128, 512], f32, name="pv",
                                           tag="mm")
                            for c in range(8):
                                nc.tensor.matmul(
                                    pv[:],
                                    xt[:, 512 * c + 128 * sp:
                                       512 * c + 128 * (sp + 1)],
                                    wv_sb[c][:, 512 * n:512 * (n + 1)],
                                    start=(c == 0),
                                    stop=(c == 7 and not bias_v))
                            if bias_v:
                                nc.tensor.matmul(
                                    pv[:], ones_row[:],
                                    bv_sb[:, 512 * n:512 * (n + 1)],
                                    start=False, stop=True)
                            nc.scalar.copy(vs[:, 512 * n:512 * (n + 1)],
                                           pv[:])
                        v_t.append(vs)

                    if stages < 4:
                        osb_stub = lnp.tile([128, H], f32, name="osb")
                        nc.vector.tensor_copy(osb_stub[:, 0:512], al_t[0][:])
                        nc.vector.tensor_copy(osb_stub[:, 512:1024],
                                              v_t[3][:, 0:512])
                        nc.sync.dma_start(out_d[4 * g:4 * (g + 1)],
                                          osb_stub[:])
                        return

                    # 7. attn.V -> rt_c [128 hd (2 heads), 128 (s, q)]
                    # pav is a full PSUM bank: a half-bank tile would share
                    # its physical bank with the pool's other rotation buf,
                    # and PE-write + DVE-read of one bank is a fatal HW
                    # PSUM collision (not modeled by CoreSim).
                    rt_t = []
                    for c in range(8):
                        pav = ppav.tile([128, 512], f32, name="pav",
                                        tag="av")
                        for sp in range(4):
                            nc.tensor.matmul(
                                pav[:, 64 * sp:64 * (sp + 1)],
                                v_t[sp][:, 128 * c:128 * (c + 1)],
                                al_t[sp][:, 64 * c:64 * (c + 1)],
                                start=True, stop=True)
                        rt = smp.tile([128, 128], bf16, name=f"rt{c}")
                        nc.vector.tensor_copy(
                            rt[0:64, :].rearrange("p (s q) -> p s q", q=32),
                            pav[0:64, 0:256]
                            .rearrange("p (s q2) -> p s q2", q2=64)[:, :, 0:32])
                        nc.vector.tensor_copy(
                            rt[64:128, :].rearrange("p (s q) -> p s q", q=32),
                            pav[64:128, 0:256]
                            .rearrange("p (s q2) -> p s q2", q2=64)[:, :, 32:64])
                        rt_t.append(rt)

                    if stages < 6:
                        osb_stub = lnp.tile([128, H], f32, name="osb")
                        nc.vector.tensor_copy(osb_stub[:, 0:64],
                                              rt_t[0][:].bitcast(f32))
                        nc.vector.tensor_copy(osb_stub[:, 64:128],
                                              rt_t[7][:].bitcast(f32))
                        nc.vector.tensor_copy(osb_stub[:, 128:256],
                                              osb_stub[:, 0:128])
                        nc.vector.tensor_copy(osb_stub[:, 256:512],
                                              osb_stub[:, 0:256])
                        nc.vector.tensor_copy(osb_stub[:, 512:1024],
                                              osb_stub[:, 0:512])
                        nc.sync.dma_start(out_d[4 * g:4 * (g + 1)],
                                          osb_stub[:])
                        return

                    # 8. O-proj: rows (s, q) on partitions, H on free
                    osb = lnp.tile([128, H], f32, name="osb")
                    for n in range(2):
                        po = ppmm.tile([128, 512], f32, name="po", tag="mm")
                        for c in range(8):
                            nc.tensor.matmul(
                                po[:], rt_t[c][:],
                                wo_sb[c][:, 512 * n:512 * (n + 1)],
                                start=(c == 0),
                                stop=(c == 7 and not bias_o))
                        if bias_o:
                            nc.tensor.matmul(
                                po[:], ones_row[:],
                                bo_sb[:, 512 * n:512 * (n + 1)],
                                start=False, stop=True)
                        nc.scalar.copy(osb[:, 512 * n:512 * (n + 1)], po[:])

                    # 9. LayerNorm over H (in place on osb).  rstd via the
                    # DVE pow ALU op, so ACT only ever needs exp+copy (one
                    # act-table load, hoisted out of the loop).
                    s1 = lnp.tile([128, 1], f32, name="s1")
                    nc.vector.tensor_reduce(s1[:], osb[:], axis=AXX, op=ADD)
                    mean = lnp.tile([128, 1], f32, name="mean")
                    nc.vector.tensor_scalar(mean[:], s1[:], 1.0 / H, None,
                                            MULT)
                    nc.vector.tensor_scalar(osb[:], osb[:], mean[:], None,
                                            SUB)
                    sq = lnp.tile([128, H], f32, name="sq")
                    nc.vector.tensor_tensor(sq[:], osb[:], osb[:], MULT)
                    ssq = lnp.tile([128, 1], f32, name="ssq")
                    nc.vector.tensor_reduce(ssq[:], sq[:], axis=AXX, op=ADD)
                    # ln(ssq/H + eps) via the activation's scale+bias, then
                    # rstd = exp(-0.5 ln(var+eps)); both funcs live in the
                    # pinned act table set.
                    lnv = lnp.tile([128, 1], f32, name="lnv")
                    nc.scalar.activation(lnv[:], ssq[:], LN_F,
                                         bias=eps_sb[:], scale=1.0 / H)
                    rstd = lnp.tile([128, 1], f32, name="rstd")
                    nc.scalar.activation(rstd[:], lnv[:], EXP, scale=-0.5)
                    nc.vector.tensor_scalar(osb[:], osb[:], rstd[:], None,
                                            MULT)
                    if gamma_beta:
                        nc.vector.tensor_tensor(osb[:], osb[:], gam_sb[:],
                                                MULT)
                        nc.vector.tensor_tensor(osb[:], osb[:], bet_sb[:],
                                                ADD)

                    # 10. out[4g:4g+4, :, :] <- rows (s-major, q); fully
                    # contiguous 512 KB store.  Issued via SWDGE (Pool
                    # queue) so the next chunk's xt load on SP never queues
                    # behind it.
                    nc.gpsimd.dma_start(out_d[4 * g:4 * (g + 1)], osb[:])

                def emit_all():
                    for g in range(NG):
                        emit_chunk(g)

                if loop > 1:
                    with tc.For_i(0, loop, 1):
                        emit_all()
                else:
                    emit_all()

    # Pin exp/ln/copy activations to the one act-table set that holds all
    # of them ("natural_log_exp_and_others") so the table load is emitted
    # once and hoisted out of the loop instead of swapping every chunk.
    import concourse.bacc as bacc_mod
    _orig_gat = bacc_mod.get_activation_tables
    _pin = {mybir.ActivationFunctionType.Exp, mybir.ActivationFunctionType.Ln,
            mybir.ActivationFunctionType.Copy,
            mybir.ActivationFunctionType.Identity}

    def _gat(arch):
        tables = _orig_gat(arch)
        return {name: (funcs if name == "natural_log_exp_and_others"
                       else funcs - _pin)
                for name, funcs in tables.items()}

    bacc_mod.get_activation_tables = _gat
    try:
        nc.compile()
    finally:
        bacc_mod.get_activation_tables = _orig_gat
    return nc


def _get(loop=1, bias_kq=False, bias_v=False, bias_o=False,
         gamma_beta=False, stages=9):
    key = (loop, bias_kq, bias_v, bias_o, gamma_beta, stages)
    if key not in _BUILD_CACHE:
        _BUILD_CACHE[key] = _build(bias_kq=bias_kq, bias_v=bias_v,
                                   bias_o=bias_o, gamma_beta=gamma_beta,
                                   loop=loop, stages=stages)
    return _BUILD_CACHE[key]


_PREP_CACHE = {}


def _prep_fns():
    """jitted CPU preprocessing (transpose/cast are multithreaded in XLA)."""
    if _PREP_CACHE:
        return _PREP_CACHE
    import jax
    import jax.numpy as jnp

    cpu = jax.devices("cpu")[0]

    def _xkt(x):  # [S*T, H] f32 -> [16, 128, 8, 512] bf16
        x4 = x.reshape(NG, 512, 8, 128)
        return x4.transpose(0, 3, 2, 1).astype(jnp.bfloat16)

    def _qkt(ini_q, Wq, bq, Wk):  # -> [8, 128, HQ] bf16
        q = ini_q @ Wq.T + bq                      # [Q, H]
        qh = q.reshape(Q, HEADS, D)
        qk = jnp.einsum("qhd,hdH->hqH", qh,
                        Wk.reshape(HEADS, D, H)) * np.float32(0.125)
        qkt = qk.reshape(HQ, H).T                  # [H, HQ]
        return qkt.reshape(8, 128, HQ).astype(jnp.bfloat16)

    def _bkq(ini_q, Wq, bq, bk):  # -> [1, HQ] bf16
        q = ini_q @ Wq.T + bq
        qh = q.reshape(Q, HEADS, D)
        t2 = jnp.einsum("qhd,hd->hq", qh,
                        bk.reshape(HEADS, D)) * np.float32(0.125)
        return t2.reshape(1, HQ).astype(jnp.bfloat16)

    _PREP_CACHE["xkt"] = jax.jit(_xkt, device=cpu)
    _PREP_CACHE["qkt"] = jax.jit(_qkt, device=cpu)
    _PREP_CACHE["bkq"] = jax.jit(_bkq, device=cpu)
    return _PREP_CACHE


def _in_maps(ini_q, ini_k, mask, Wq, bq, Wk, bk, Wv, bv, Wo, bo, gamma, beta):
    import ml_dtypes
    f = np.float32
    bfdt = ml_dtypes.bfloat16
    fns = _prep_fns()

    wvt = np.asarray(Wv, dtype=f).T.astype(bfdt)
    wot = np.asarray(Wo, dtype=f).T.astype(bfdt)
    shared = dict(
        wvt=np.ascontiguousarray(wvt),
        wot=np.ascontiguousarray(wot),
        bvr=np.asarray(bv, dtype=f).reshape(1, H).astype(bfdt),
        bor=np.asarray(bo, dtype=f).reshape(1, H).astype(bfdt),
        gam=np.asarray(gamma, dtype=f).reshape(1, H),
        bet=np.asarray(beta, dtype=f).reshape(1, H),
    )
    ini_q = np.asarray(ini_q, dtype=f)
    ini_k = np.asarray(ini_k, dtype=f)
    mask = np.asarray(mask, dtype=f)
    Wq_, bq_, Wk_, bk_ = (np.asarray(a, dtype=f) for a in (Wq, bq, Wk, bk))
    maps = []
    for b in range(B):
        m = dict(shared)
        m["xkt"] = np.asarray(fns["xkt"](ini_k[b].reshape(ST, H)))
        m["qkt"] = np.asarray(fns["qkt"](ini_q[b], Wq_, bq_, Wk_))
        m["bkq"] = np.asarray(fns["bkq"](ini_q[b], Wq_, bq_, bk_))
        m["mnegt"] = np.ascontiguousarray(mask[b].T * f(-10000.0))
        maps.append(m)
    return maps


def run(inputs, loop=1, full_results=False, stages=9):
    """Run the SPMD kernel; returns (B, Q, S, H) float32."""
    from concourse.bass_utils import run_bass_kernel_spmd

    flags = dict(
        stages=stages,
        bias_kq=bool(np.any(inputs["bq"]) or np.any(inputs["bk"])),
        bias_v=bool(np.any(inputs["bv"])),
        bias_o=bool(np.any(inputs["bo"])),
        gamma_beta=bool(np.any(np.asarray(inputs["gamma"]) != 1.0)
                        or np.any(inputs["beta"])),
    )
    nc = _get(loop=loop, **flags)
    maps = _in_maps(**inputs)
    err = None
    for _ in range(4):
        try:
            res = run_bass_kernel_spmd(nc, maps, list(range(NCORES)))
        except Exception as e:  # transient NRT device errors: retry
            err = e
            import time as _t
            _t.sleep(2.0)
            continue
        if full_results:
            return res
        # device output is s-major [S, Q, H]; transpose back to [Q, S, H]
        out = np.stack([res.results[c]["out"].transpose(1, 0, 2)
                        for c in range(NCORES)], axis=0)
        # transient first-execution corruption has been observed once on
        # this fleet; non-finite output -> re-execute
        if np.isfinite(out).all():
            return out
        err = RuntimeError("non-finite kernel output")
    raise err


def kernel(**inputs):
    return run(inputs, loop=1)
